# revision 1
# baseline (speedup 1.0000x reference)
"""Trainium2 Bass kernel for nn_MultiHeadAttention_79130477461654.

The reference einsum "nhqk,nhvd->nhqd" contracts k and v independently, so
out = (sum_k softmax(energy))*(sum_s v) = broadcast(sum_s v) since softmax
rows sum to 1.  With v = split_heads(x @ Wv) and the reference's direct
(n,h,q,d)->(n,s,e) reshape, the full output reduces to

    xs[n]    = sum_s x[n,s,:]                       (1024,)
    Z[n]     = xs[n] @ Wv                           (1024,)
    WoSum    = sum_m Wo[64m+d, :]  (d=0..63)        (64, 1024)
    T[n,h,:] = Z[n][64h:64h+64] @ WoSum + bo        (16, 1024)
    out[n, 64h+r, :] = T[n,h,:]   for r in 0..63

Sharding: data parallel over batch N=8, one batch per core; Wv/Wo
replicated.  All arithmetic on-device.

x/Wv/Wo are staged to DRAM as bf16 (host-side cast, well inside the 2e-2
tolerance), halving the input stream to ~6 MB/core - the stream is the
HBM-bound critical path.  The x seq-sum still accumulates in fp32 on DVE.
Output is written as bf16 via broadcast-DMAs (stride-0 repeat dim) from
all 128 partitions (T replicated to rows m = 8h + rr so DRAM row 8m + r8
is affine in the partition index).
"""

import numpy as np

N, S, E, H, D = 8, 1024, 1024, 16, 64
NCORES = 8
P = 128  # partitions


def build_nc():
    import concourse.bacc as bacc
    import concourse.mybir as mybir
    from concourse.tile import TileContext

    F32 = mybir.dt.float32
    BF16 = mybir.dt.bfloat16
    nc = bacc.Bacc("TRN2", target_bir_lowering=False, debug=False)

    xtd = nc.declare_dram_parameter("xT", [E, S], F32, isOutput=False)
    wvd = nc.declare_dram_parameter("Wv", [E, E], BF16, isOutput=False)
    wod = nc.declare_dram_parameter("Wo", [E, E], BF16, isOutput=False)
    bod = nc.declare_dram_parameter("bo1", [1, E], F32, isOutput=False)
    i2d = nc.declare_dram_parameter("I2", [D, P], BF16, isOutput=False)
    oned = nc.declare_dram_parameter("one1", [1, 1], F32, isOutput=False)
    outd = nc.declare_dram_parameter("out", [S, E], BF16, isOutput=True)

    # two HWDGE queues: SP (sync) and ACT (scalar)
    dmae = [nc.sync, nc.scalar]

    with TileContext(nc) as tc:
        with (
            tc.tile_pool(name="xin", bufs=4) as xp,
            tc.tile_pool(name="wv", bufs=4) as wvp,
            tc.tile_pool(name="wo", bufs=2) as wop,
            tc.tile_pool(name="small", bufs=1) as sp,
            tc.tile_pool(name="psZ", bufs=1, space="PSUM") as psZ,
            tc.tile_pool(name="psS", bufs=1, space="PSUM") as psS,
            tc.tile_pool(name="psY", bufs=1, space="PSUM") as psY,
            tc.tile_pool(name="psT", bufs=1, space="PSUM") as psT,
        ):
            # tiny consts on the SWDGE queue so the HWDGE queues stream x at once
            one_sb = sp.tile([1, 1], F32)
            nc.gpsimd.dma_start(out=one_sb[:], in_=oned[:])
            i2_sb = sp.tile([D, P], BF16)
            nc.gpsimd.dma_start(out=i2_sb[:], in_=i2d[:])
            bo_sb = sp.tile([P, E], F32)

            # ---- input DMAs per queue: x as two fp32 1MB pair-tiles, Wv as
            #      two bf16 512KB pair-tiles, Wo as one bf16 1MB quad.
            #      Queue q carries e/row blocks 4q..4q+3.
            xr = xtd.rearrange("(q j c p) s -> q j p c s", p=P, c=2, j=2)
            wr = wvd.rearrange("(q j c p) e -> q j p c e", p=P, c=2, j=2)
            wor = wod.rearrange("(q c p) e -> q p c e", p=P, c=4)
            xts = [[None] * 2 for _ in range(2)]
            wvt = [[None] * 2 for _ in range(2)]
            wot = [None] * 2
            for j in range(2):
                for q in range(2):
                    t = xp.tile([P, 2 * S], F32, tag="xt")
                    dmae[q].dma_start(
                        out=t[:].rearrange("p (c s) -> p c s", c=2), in_=xr[q, j]
                    )
                    xts[q][j] = t
            for j in range(2):
                for q in range(2):
                    t = wvp.tile([P, 2 * E], BF16, tag="wv")
                    dmae[q].dma_start(
                        out=t[:].rearrange("p (c e) -> p c e", c=2), in_=wr[q, j]
                    )
                    wvt[q][j] = t
            for q in range(2):
                t = wop.tile([P, 4 * E], BF16, tag="wo")
                dmae[q].dma_start(
                    out=t[:].rearrange("p (c e) -> p c e", c=4), in_=wor[q]
                )
                wot[q] = t
            # bias row replicated to all 128 partitions by the DMA itself;
            # queued after wo (needed only at the bias add) so it does not
            # steal SDMA round-robin share from the x stream
            dmae[0].dma_start(out=bo_sb[:], in_=bod[0:1, :].to_broadcast((P, E)))

            # ---- DVE: x reduces in arrival order (fp32 accumulate), then
            #      bf16 copy.  xpT[p, k] = sum_s x[128k+p, s]
            xpT = sp.tile([P, 8], F32)
            xpb = sp.tile([P, 8], BF16)
            for j in range(2):
                for q in range(2):
                    k0 = 4 * q + 2 * j
                    nc.vector.tensor_reduce(
                        xpT[:, k0 : k0 + 2],
                        xts[q][j][:].rearrange("p (c s) -> p c s", c=2),
                        axis=mybir.AxisListType.X,
                        op=mybir.AluOpType.add,
                    )
                    # per-reduce bf16 slice copy so early z matmuls don't
                    # wait for the last reduce
                    nc.vector.tensor_copy(xpb[:, k0 : k0 + 2], xpT[:, k0 : k0 + 2])

            # ---- Z row (1, 1024) = xs @ Wv  (bf16, chases the wv DMAs)
            ps_z = psZ.tile([1, E], F32, tag="psz")
            order = [(0, 0), (1, 0), (0, 1), (1, 1)]
            for idx, (q, j) in enumerate(order):
                for c in range(2):
                    k = 4 * q + 2 * j + c
                    for half in range(2):
                        sl = slice(half * 512, half * 512 + 512)
                        nc.tensor.matmul(
                            ps_z[0:1, sl],
                            xpb[:, k : k + 1],
                            wvt[q][j][:, c * E + half * 512 : c * E + half * 512 + 512],
                            start=(idx == 0 and c == 0),
                            stop=(idx == 3 and c == 1),
                            skip_group_check=True,
                        )
            # fp32 on the (1, 1024) row: a single-partition fp32->bf16 CAST is
            # element-serial on one DVE lane (~5.7us); the fp32 copy is ~1.2us
            # and the K=1 fp32 matmuls stay warm behind the z chain
            srow = sp.tile([1, E], F32)
            nc.vector.tensor_copy(srow[0:1, 0:512], ps_z[0:1, 0:512])
            nc.vector.tensor_copy(srow[0:1, 512:E], ps_z[0:1, 512:E])

            # ---- sft[d, h] = Z[64h + d]  (rank-1 bf16 matmuls, K=1), then
            #      sft8 free-dim broadcast, then dup matmul -> YTx8 (128, 128)
            #      with rows m = 8h + rr  (I2[k,m]=1 iff m%64==k)
            ps_sft = psS.tile([D, H], F32, tag="pss")
            for h in range(H):
                nc.tensor.matmul(
                    ps_sft[:, h : h + 1],
                    srow[0:1, h * D : (h + 1) * D],
                    one_sb[0:1, 0:1],
                    start=True,
                    stop=True,
                )
            sft8 = sp.tile([D, P], BF16)
            nc.vector.tensor_copy(
                sft8[:].rearrange("d (h rr) -> d h rr", rr=8),
                ps_sft[:, :, None].to_broadcast((D, H, 8)),
            )
            ps_ytx = psY.tile([P, P], F32, tag="psy")
            nc.tensor.matmul(ps_ytx[:], i2_sb[:], sft8[:], start=True, stop=True)
            ytx8 = sp.tile([P, P], BF16)
            nc.vector.tensor_copy(ytx8[:], ps_ytx[:])

            # ---- w128[p, :] = sum_rb Wo[128 rb + p, :]: bf16 DVE folds
            wq0 = sp.tile([P, 2 * E], BF16)
            wq1 = sp.tile([P, 2 * E], BF16)
            w128 = sp.tile([P, E], BF16)
            nc.vector.tensor_add(wq0[:], wot[0][:, 0 : 2 * E], wot[0][:, 2 * E :])
            nc.vector.tensor_add(wq1[:], wot[1][:, 0 : 2 * E], wot[1][:, 2 * E :])

            # ---- tail per j-quarter: final folds, T8 matmul, bias+cast,
            #      broadcast store out[8m + r8, :] = T8[m, :] = T[m//8, :]
            ps_t = psT.tile([P, E], F32, tag="pst")
            tb8 = sp.tile([P, E], BF16)
            outr = outd.rearrange("(m r8) j -> m r8 j", r8=8)
            for qt in range(4):
                sl = slice(qt * 256, qt * 256 + 256)
                nc.vector.tensor_add(w128[:, sl], wq0[:, sl], wq0[:, E + qt * 256 : E + qt * 256 + 256])
                nc.vector.tensor_add(w128[:, sl], w128[:, sl], wq1[:, sl])
                nc.vector.tensor_add(w128[:, sl], w128[:, sl], wq1[:, E + qt * 256 : E + qt * 256 + 256])
                nc.tensor.matmul(
                    ps_t[:, sl], ytx8[:], w128[:, sl], start=True, stop=True,
                    skip_group_check=True,
                )
                nc.vector.tensor_add(tb8[:, sl], ps_t[:, sl], bo_sb[:, sl])
                dmae[qt % 2].dma_start(
                    out=outr[:, :, sl],
                    in_=tb8[:, None, sl].to_broadcast((P, 8, 256)),
                )

    nc.compile()
    return nc


_NC_CACHE = None


def make_in_maps(x, Wv, Wo, bo):
    import ml_dtypes

    BF = ml_dtypes.bfloat16
    x = np.ascontiguousarray(np.asarray(x, dtype=np.float32))
    Wv = np.ascontiguousarray(np.asarray(Wv, dtype=np.float32).astype(BF))
    Wo = np.ascontiguousarray(np.asarray(Wo, dtype=np.float32).astype(BF))
    bo = np.ascontiguousarray(np.asarray(bo, dtype=np.float32))
    bo1 = bo.reshape(1, E)
    I2 = np.zeros((D, P), dtype=BF)
    I2[np.arange(P) % D, np.arange(P)] = 1.0
    one1 = np.ones((1, 1), dtype=np.float32)
    return [
        {
            "xT": np.ascontiguousarray(x[j].T),
            "Wv": Wv,
            "Wo": Wo,
            "bo1": bo1,
            "I2": I2,
            "one1": one1,
        }
        for j in range(NCORES)
    ]


def kernel(x, Wq=None, Wk=None, Wv=None, Wo=None, bo=None, **_unused):
    from concourse.bass_utils import run_bass_kernel_spmd

    global _NC_CACHE
    if _NC_CACHE is None:
        _NC_CACHE = build_nc()
    nc = _NC_CACHE

    in_maps = make_in_maps(x, Wv, Wo, bo)
    res = run_bass_kernel_spmd(nc, in_maps, core_ids=list(range(NCORES))).results
    return np.stack(
        [res[j]["out"].astype(np.float32) for j in range(NCORES)], axis=0
    )



# revision 3
# speedup vs baseline: 1.2499x; 1.2499x over previous
"""Trainium2 Bass kernel for nn_MultiHeadAttention_79130477461654.

The reference einsum "nhqk,nhvd->nhqd" contracts k and v independently, so
out = (sum_k softmax(energy))*(sum_s v) = broadcast(sum_s v) since softmax
rows sum to 1.  With v = split_heads(x @ Wv) and the reference's direct
(n,h,q,d)->(n,s,e) reshape, the full output reduces to

    xs[n]    = sum_s x[n,s,:]                       (1024,)
    Z[n]     = xs[n] @ Wv                           (1024,)
    WoSum    = sum_m Wo[64m+d, :]  (d=0..63)        (64, 1024)
    T[n,h,:] = Z[n][64h:64h+64] @ WoSum + bo        (16, 1024)
    out[n, 64h+r, :] = T[n,h,:]   for r in 0..63

Sharding: data parallel over batch N=8, one batch per core; weights
replicated.  All arithmetic on-device; host only casts dtypes and
re-lays-out tensors.

v2 layout/schedule (vs the 51 us v1):
  - x staged bf16 (was fp32): input stream 8.4 MB -> 6.3 MB/core.
  - stream order per HWDGE queue interleaves x-chunk k with Wv-chunk k so
    the Z accumulation (and PE warmth) chases the stream.
  - Wo is streamed LAST as two host-contiguous column-halves; the WoSum
    row-fold is fused into the T matmul as a PSUM accumulation over the 8
    row-blocks (T8 = sum_rb ytx8 @ Wo[128rb:128rb+128, half]), so the
    tail transpose-dance hides under the Wo stream and T/out halves
    pipeline against it.
  - bias enters the same PSUM group as a K=1 matmul (ones x bo), removing
    the broadcast-bias DMA and the DVE bias add.
  - Z -> srow copies run on the Scalar (ACT) engine in bf16 so the 16
    rank-1 transpose matmuls are single-pass bf16 (v1: fp32 LOW_HIGH).
  - out written as two 1 MiB column-half broadcast DMAs (1 KB descrs),
    issued as each T half completes.
"""

import numpy as np

N, S, E, H, D = 8, 1024, 1024, 16, 64
NCORES = 8
P = 128  # partitions
NCHUNK = 8  # 1024 rows / 128


def build_nc():
    import concourse.bacc as bacc
    import concourse.mybir as mybir
    from concourse.tile import TileContext

    F32 = mybir.dt.float32
    BF16 = mybir.dt.bfloat16
    nc = bacc.Bacc("TRN2", target_bir_lowering=False, debug=False)

    xtd = nc.declare_dram_parameter("xT", [E, S], BF16, isOutput=False)
    wvd = nc.declare_dram_parameter("Wv", [E, E], BF16, isOutput=False)
    # Wo re-laid-out on host as two contiguous column halves: [2048, 512]
    wod = nc.declare_dram_parameter("WoH", [2 * E, E // 2], BF16, isOutput=False)
    bod = nc.declare_dram_parameter("bo1", [1, E], BF16, isOutput=False)
    i2d = nc.declare_dram_parameter("I2", [D, P], BF16, isOutput=False)
    outd = nc.declare_dram_parameter("out", [S, E], BF16, isOutput=True)

    # two HWDGE queues: SP (sync) and ACT (scalar)
    dmae = [nc.sync, nc.scalar]

    with TileContext(nc) as tc:
        with (
            tc.tile_pool(name="xin", bufs=NCHUNK) as xp,
            tc.tile_pool(name="wv", bufs=NCHUNK) as wvp,
            tc.tile_pool(name="wo", bufs=4) as wop,
            tc.tile_pool(name="small", bufs=1) as sp,
            tc.tile_pool(name="psZ", bufs=1, space="PSUM") as psZ,
            tc.tile_pool(name="psS", bufs=1, space="PSUM") as psS,
            tc.tile_pool(name="psY", bufs=1, space="PSUM") as psY,
            tc.tile_pool(name="psT", bufs=1, space="PSUM") as psT,
        ):
            # tiny consts on the SWDGE queue so the HWDGE queues stream at once
            i2_sb = sp.tile([D, P], BF16)
            nc.gpsimd.dma_start(out=i2_sb[:], in_=i2d[:])
            bo_sb = sp.tile([1, E], BF16)
            nc.gpsimd.dma_start(out=bo_sb[:], in_=bod[:])
            ones18 = sp.tile([1, 8], BF16)
            nc.vector.memset(ones18[:], 1.0)
            ones128 = sp.tile([1, P], BF16)
            nc.vector.memset(ones128[:], 1.0)

            # ---- input DMAs: x/Wv as 8 256KB chunk tiles each, chunk k of
            #      x and Wv adjacent on queue k%2 so Z-chunk matmuls fire
            #      throughout the stream; Wo last as 4 512KB quarter tiles
            #      (two column-halves, each split into rb-groups 0-3 / 4-7).
            xr = xtd.rearrange("(k p) s -> k p s", p=P)
            wr = wvd.rearrange("(k p) e -> k p e", p=P)
            wor = wod.rearrange("(t rb p) c -> t p rb c", rb=4, p=P)
            xts = [None] * NCHUNK
            wvt = [None] * NCHUNK
            for k in range(NCHUNK):
                q = k % 2
                t = xp.tile([P, S], BF16, tag="xt")
                dmae[q].dma_start(out=t[:], in_=xr[k])
                xts[k] = t
                t = wvp.tile([P, E], BF16, tag="wv")
                dmae[q].dma_start(out=t[:], in_=wr[k])
                wvt[k] = t
            wot = [None] * 4
            for i in range(4):
                # i = 0,1 -> column half A (rb 0-3, 4-7); i = 2,3 -> half B.
                # halves split across both queues so half A lands first.
                t = wop.tile([P, 4 * (E // 2)], BF16, tag="wo")
                dmae[i % 2].dma_start(
                    out=t[:].rearrange("p (rb c) -> p rb c", rb=4), in_=wor[i]
                )
                wot[i] = t

            # ---- DVE: per-chunk seq-sum of x straight to bf16 (fp32
            #      internal accumulation on DVE): xpb[p, k] = sum_s x[128k+p, s]
            xpb = sp.tile([P, NCHUNK], BF16)
            with nc.allow_low_precision(
                reason="DVE reduce accumulates fp32 internally; bf16 only on write"
            ):
                for k in range(NCHUNK):
                    nc.vector.tensor_reduce(
                        xpb[:, k : k + 1],
                        xts[k][:],
                        axis=mybir.AxisListType.X,
                        op=mybir.AluOpType.add,
                    )

            # ---- Z row (1, 1024) = xs @ Wv, accumulated chunk by chunk as
            #      the stream delivers (x_k, Wv_k); bf16 single-pass.
            ps_z = psZ.tile([1, E], F32, tag="psz")
            for k in range(NCHUNK):
                for half in range(2):
                    sl = slice(half * 512, half * 512 + 512)
                    nc.tensor.matmul(
                        ps_z[0:1, sl],
                        xpb[:, k : k + 1],
                        wvt[k][:, sl],
                        start=(k == 0),
                        stop=(k == NCHUNK - 1),
                        skip_group_check=True,
                    )

            # ---- bias rows into the T PSUM group first (bo arrives early):
            #      psT[m, e'] starts at bo[e'] via K=1 matmul ones128 x bo
            ps_t = psT.tile([P, E], F32, tag="pst")
            for half in range(2):
                sl = slice(half * 512, half * 512 + 512)
                nc.tensor.matmul(
                    ps_t[:, sl],
                    ones128[0:1, :],
                    bo_sb[0:1, sl],
                    start=True,
                    stop=False,
                    skip_group_check=True,
                )

            # ---- Z -> srow (bf16, on ACT so DVE stays free and the rank-1
            #      transposes below run single-pass bf16)
            srow = sp.tile([1, E], BF16)
            for half in range(2):
                sl = slice(half * 512, half * 512 + 512)
                nc.scalar.activation(
                    srow[0:1, sl], ps_z[0:1, sl],
                    func=mybir.ActivationFunctionType.Copy,
                )

            # ---- transpose dance: ps_sft[d, 8h+rr] = Z[64h+d] via 16
            #      rank-1 matmuls (rhs = ones[1,8] replicates over rr)
            ps_sft = psS.tile([D, P], F32, tag="pss")
            for h in range(H):
                nc.tensor.matmul(
                    ps_sft[:, 8 * h : 8 * h + 8],
                    srow[0:1, h * D : (h + 1) * D],
                    ones18[0:1, :],
                    start=True,
                    stop=True,
                    skip_group_check=True,
                )
            sft8 = sp.tile([D, P], BF16)
            nc.vector.tensor_copy(sft8[:], ps_sft[:])
            # dup matmul: ytx8[p, m] = sft8[p%64, m]  (I2[d,p]=1 iff d==p%64)
            ps_ytx = psY.tile([P, P], F32, tag="psy")
            nc.tensor.matmul(
                ps_ytx[:], i2_sb[:], sft8[:], start=True, stop=True,
                skip_group_check=True,
            )
            ytx8 = sp.tile([P, P], BF16)
            nc.vector.tensor_copy(ytx8[:], ps_ytx[:])

            # ---- T accumulation fused with the Wo row-fold: for column
            #      half, psT[:, half] += sum_rb ytx8 @ Wo[128rb+p, half].
            #      Chases the Wo stream tile by tile; then bf16 copy and the
            #      broadcast store out[8m + r8, half] = tb8[m, half].
            tb8 = sp.tile([P, E], BF16)
            outr = outd.rearrange("(m r8) e -> m r8 e", r8=8)
            for half in range(2):
                sl = slice(half * 512, half * 512 + 512)
                for i in (0, 1):
                    wt = wot[2 * half + i]
                    for rb in range(4):
                        nc.tensor.matmul(
                            ps_t[:, sl],
                            ytx8[:],
                            wt[:, rb * 512 : rb * 512 + 512],
                            start=False,
                            stop=(i == 1 and rb == 3),
                            skip_group_check=True,
                        )
                nc.vector.tensor_copy(tb8[:, sl], ps_t[:, sl])
                dmae[half].dma_start(
                    out=outr[:, :, sl],
                    in_=tb8[:, None, sl].to_broadcast((P, 8, 512)),
                )

    nc.compile()
    return nc


_NC_CACHE = None


def make_in_maps(x, Wv, Wo, bo):
    import ml_dtypes

    BF = ml_dtypes.bfloat16
    x = np.asarray(x, dtype=np.float32)
    Wv = np.ascontiguousarray(np.asarray(Wv, dtype=np.float32).astype(BF))
    Wo = np.asarray(Wo, dtype=np.float32).astype(BF)
    WoH = np.ascontiguousarray(np.concatenate([Wo[:, :512], Wo[:, 512:]], axis=0))
    bo1 = np.asarray(bo, dtype=np.float32).astype(BF).reshape(1, E)
    I2 = np.zeros((D, P), dtype=BF)
    I2[np.arange(P) % D, np.arange(P)] = 1.0
    return [
        {
            "xT": np.ascontiguousarray(x[j].T.astype(BF)),
            "Wv": Wv,
            "WoH": WoH,
            "bo1": bo1,
            "I2": I2,
        }
        for j in range(NCORES)
    ]


def kernel(x, Wq=None, Wk=None, Wv=None, Wo=None, bo=None, **_unused):
    from concourse.bass_utils import run_bass_kernel_spmd

    global _NC_CACHE
    if _NC_CACHE is None:
        _NC_CACHE = build_nc()
    nc = _NC_CACHE

    in_maps = make_in_maps(x, Wv, Wo, bo)
    res = run_bass_kernel_spmd(nc, in_maps, core_ids=list(range(NCORES))).results
    return np.stack(
        [res[j]["out"].astype(np.float32) for j in range(NCORES)], axis=0
    )
